# revision 47
# baseline (speedup 1.0000x reference)
"""DBRX attention block on 8 Trainium2 NeuronCores.

Sharding: tensor-parallel over heads. Each core owns 4 query heads and the
single KV head that serves them (GQA group), computes the fused QKV
projection for its rows, clip, RoPE, causal flash-style attention, and a
full-width partial of the output projection (its 512 columns of the out-proj
contraction). The 8 partial outputs are summed on the host.

All matmuls run in bf16 (fp32 matmul is 4 cycles/row on TRN2 PE; bf16 is 1).
Softmax runs without max-subtraction (scores are O(1) for this input
distribution; exp cannot overflow), which matches the reference softmax
mathematically.

v5: the kernel is one software-pipelined pass, engine-balanced:
  R0: QKV for batch 0 (PE-bound; DMA preamble interleaved so the first
      matmul issues ~3us in).
  R1: QKV for batch 1 (2-bank psum pairs) interleaved with batch-0
      attention, whose serializing scalar-engine exp hides under the
      projection matmuls.
  R2: batch-1 attention interleaved with the output projection; out-proj
      tiles unlock progressively as query groups are normalized (g-outer
      loop), draining on whatever PE slack the exp pipeline leaves.
Other key structure:
  - causal trimming: the kt chunk at diagonal index u only computes
    q columns [128u, 512) in scores / exp / AV; the triangle mask multiply
    only touches the 128-wide partial block.
  - per-chunk softmax row-sums accumulate on the DVE (at_sum += at), one
    ones-matmul per (b, h, qgroup) turns them into denominators.
  - psum->sbuf drains alternate scalar/vector; out rows DMA once per
    128-token chunk (gpsimd has no PSUM port, so drains stay off it).

Layouts (per core):
  hidT    [KC, 128, T]        hidden states transposed, bf16
  wqkvT   [128, KC, 6, 128]   [d%128, d//128, row-block, row%128]; row blocks
                              0-3 = q heads, 4 = k head, 5 = v head
  cosT    [128, S]            rope cos, transposed (same for both batches)
  sinTs   [128, S]            rope sin, transposed, first 64 rows negated
  tri     [128, 128]          causal triangle mask, tri[p, j] = (p <= j)
  ident   [128, 128]          identity for PE transpose
  woutT   [128, 4, D]         Wout[:, core cols].T tiled by head chunk
  out     [T, 8, 512]         partial output (bf16), summed on host
"""

import sys

sys.path.insert(0, "/opt/trn_rl_repo")

import numpy as np
import ml_dtypes

import concourse.bass as bass
import concourse.tile as tile
from concourse import bacc, mybir
from contextlib import ExitStack

BF16 = mybir.dt.bfloat16
F32 = mybir.dt.float32
NPBF16 = ml_dtypes.bfloat16

# problem dims (must match reference.py / spec.json)
B, S, D = 2, 2048, 4096
NH, NKV, HD = 32, 8, 128
CLIP = 8.0
SCALE = HD**-0.5
NCORES = 8
HPC = NH // NCORES  # q heads per core

PART = 128
NTG = 512  # token-group width (phase-1 N, phase-2 qt group, phase-3 dout group)

STATS = {}


def _build_core_program(b=B, s=S, d=D, hpc=HPC, debug=False):
    """Bass program for ONE core (SPMD: same program, per-core data)."""
    t = b * s
    kc_n = d // PART  # contraction chunks
    m_n = hpc + 2  # qkv row blocks per core
    ng_n = t // NTG  # token groups
    sc_n = s // PART  # kt chunks per batch
    gq_n = s // NTG  # qt groups per batch
    dg_n = d // NTG  # out-proj dout groups
    tch_n = t // PART  # token chunks
    upg = NTG // PART  # diagonal sub-chunks per q group

    nc = bacc.Bacc()
    hidT = nc.declare_dram_parameter("hidT", [kc_n, PART, t], BF16, False)
    wqkvT = nc.declare_dram_parameter("wqkvT", [PART, kc_n, m_n, PART], BF16, False)
    cosT = nc.declare_dram_parameter("cosT", [PART, s], BF16, False)
    sinTs = nc.declare_dram_parameter("sinTs", [PART, s], BF16, False)
    tri = nc.declare_dram_parameter("tri", [PART, PART], BF16, False)
    ident = nc.declare_dram_parameter("ident", [PART, PART], BF16, False)
    woutT = nc.declare_dram_parameter("woutT", [PART, hpc, d], BF16, False)
    outp = nc.declare_dram_parameter("out", [t, dg_n, NTG], BF16, True)

    A = mybir.AluOpType
    ACT = mybir.ActivationFunctionType

    with tile.TileContext(nc) as tc, ExitStack() as ctx:
        persist = ctx.enter_context(tc.tile_pool(name="persist", bufs=1))
        qT = [persist.tile([PART, t], BF16, name=f"qT{h}", tag=f"qT{h}") for h in range(hpc)]
        kT = persist.tile([PART, t], BF16, name="kT", tag="kT")
        vsb = persist.tile([PART, tch_n, PART], BF16, name="vsb", tag="vsb")
        tri_sb = persist.tile([PART, PART], BF16, name="tri_sb", tag="tri")
        id_sb = persist.tile([PART, PART], BF16, name="id_sb", tag="ident")
        ones_sb = persist.tile([PART, 1], BF16, name="ones_sb", tag="ones")

        nc.vector.memset(ones_sb, 1.0)

        # attention pools open below the QKV pools on the pool stack so the
        # QKV pools can release mid-program (pools are strictly LIFO)
        sc_ps = ctx.enter_context(tc.tile_pool(name="scps", bufs=4, space="PSUM"))
        o_ps = ctx.enter_context(tc.tile_pool(name="ops", bufs=2, space="PSUM"))
        at_p = ctx.enter_context(tc.tile_pool(name="atp", bufs=22))
        sm_p = ctx.enter_context(tc.tile_pool(name="smp", bufs=3))
        ao_p = ctx.enter_context(tc.tile_pool(name="aop", bufs=1))
        aoTs = [
            [ao_p.tile([PART, s], BF16, name=f"aoT0{h}", tag=f"aoT0{h}") for h in range(hpc)],
            None,  # batch 1 tiles allocated in R2, after QKV pools release
        ]

        # ---------------- QKV machinery
        p1 = ExitStack()  # closed after R1 (frees weights/tables/activations)
        wp = p1.enter_context(tc.tile_pool(name="wp", bufs=1))
        wq_sb = wp.tile([PART, kc_n, m_n, PART], BF16, name="wq_sb", tag="wq")
        cs = p1.enter_context(tc.tile_pool(name="cs", bufs=1))
        cos_sb = cs.tile([PART, s], BF16, name="cos_sb", tag="cos")
        sin_sb = cs.tile([PART, s], BF16, name="sin_sb", tag="sin")
        hid_pool = p1.enter_context(tc.tile_pool(name="hidp", bufs=5))
        ev = p1.enter_context(tc.tile_pool(name="ev", bufs=3))

        KCB = 4  # hid quarter-tiles per token group
        kcw = kc_n // KCB

        def ht_quarter(t0, kb):
            q = hid_pool.tile([PART, kcw, NTG], BF16, name="ht", tag="ht")
            nc.sync.dma_start(
                out=q,
                in_=hidT[kb * kcw : (kb + 1) * kcw, :, t0 : t0 + NTG].transpose(
                    [1, 0, 2]
                ),
            )
            return q

        def clip_ps(ps):
            xc = ev.tile([PART, NTG], BF16, name="xc", tag="xc", bufs=5)
            nc.vector.tensor_scalar(
                out=xc, in0=ps, scalar1=CLIP, scalar2=-CLIP, op0=A.min, op1=A.max
            )
            return xc

        def do_v(xc, ng, pool, tag):
            for u in range(upg):
                tp = pool.tile([PART, PART], BF16, name="tp", tag=tag)
                nc.tensor.transpose(tp, xc[:, u * PART : (u + 1) * PART], id_sb)
                nc.scalar.copy(out=vsb[:, ng * upg + u, :], in_=tp)

        def do_rope(xc, m, t0):
            tl = t0 % s  # cos/sin tables are per-batch
            rot = ev.tile([PART, NTG], BF16, name="rot", tag="rot")
            hh = PART // 2
            nc.gpsimd.dma_start(out=rot[0:hh, :], in_=xc[hh:PART, :])
            nc.gpsimd.dma_start(out=rot[hh:PART, :], in_=xc[0:hh, :])
            t1 = ev.tile([PART, NTG], BF16, name="t1", tag="t1")
            nc.vector.tensor_tensor(
                out=t1, in0=xc, in1=cos_sb[:, tl : tl + NTG], op=A.mult
            )
            t2 = ev.tile([PART, NTG], BF16, name="t2", tag="t2")
            nc.vector.tensor_tensor(
                out=t2, in0=rot, in1=sin_sb[:, tl : tl + NTG], op=A.mult
            )
            dest = kT if m == m_n - 2 else qT[m]
            nc.vector.tensor_tensor(
                out=dest[:, t0 : t0 + NTG], in0=t1, in1=t2, op=A.add
            )

        # -------- preamble: first token group's activations interleaved with
        # weights so the first matmul issues ~3us in; tables land last
        # three parallel preamble DMA streams: activations on the sync
        # queue, weights (v/k rows first) on gpsimd, tables on scalar
        nc.gpsimd.dma_start(out=wq_sb[:, :, hpc:, :], in_=wqkvT[:, :, hpc:, :])
        hts0 = [ht_quarter(0, kb) for kb in range(KCB)]
        nc.gpsimd.dma_start(out=wq_sb[:, :, 0:2, :], in_=wqkvT[:, :, 0:2, :])
        nc.gpsimd.dma_start(out=wq_sb[:, :, 2:hpc, :], in_=wqkvT[:, :, 2:hpc, :])
        nc.scalar.dma_start(out=tri_sb, in_=tri[:, :])
        nc.scalar.dma_start(out=id_sb, in_=ident[:, :])
        nc.scalar.dma_start(out=cos_sb, in_=cosT[:, :])
        nc.scalar.dma_start(out=sin_sb, in_=sinTs[:, :])

        def col_off(kt, g):
            u = kt - g * upg
            return u * PART if u > 0 else 0

        def emit_A(bb, h, g):
            """scores + exp + mask + running row-total for one q group"""
            q0 = bb * s + g * NTG
            nk = (g + 1) * upg
            ats = []
            at_sum = sm_p.tile([PART, NTG], BF16, name="at_sum", tag="at_sum", bufs=3)
            for kt in range(nk):
                o = col_off(kt, g)
                scp = sc_ps.tile([PART, NTG], F32, name="scp", tag="scp")
                nc.tensor.matmul(
                    scp[:, o:NTG],
                    lhsT=kT[:, bb * s + kt * PART : bb * s + (kt + 1) * PART],
                    rhs=qT[h][:, q0 + o : q0 + NTG],
                    start=True,
                    stop=True,
                )
                at = at_p.tile([PART, NTG], BF16, name="at", tag="at")
                nc.scalar.activation(
                    out=at[:, o:NTG], in_=scp[:, o:NTG], func=ACT.Exp, scale=SCALE
                )
                u = kt - g * upg
                if u >= 0:  # diagonal chunk: triangle on its partial block
                    nc.vector.tensor_tensor(
                        out=at[:, o : o + PART],
                        in0=at[:, o : o + PART],
                        in1=tri_sb,
                        op=A.mult,
                    )
                if kt == 0:
                    nc.vector.tensor_copy(out=at_sum, in_=at)
                else:
                    nc.vector.tensor_tensor(
                        out=at_sum[:, o:NTG],
                        in0=at_sum[:, o:NTG],
                        in1=at[:, o:NTG],
                        op=A.add,
                    )
                ats.append(at)
            return (bb, h, g, q0, nk, ats, at_sum)

        def emit_B(bb, h, g, q0, nk, ats, at_sum):
            """AV accumulation + single row-sum matmul + reciprocal"""
            op = o_ps.tile([PART, NTG], F32, name="op", tag="op")
            for kt in range(nk):
                o = col_off(kt, g)
                nc.tensor.matmul(
                    op[:, o:NTG],
                    lhsT=vsb[:, bb * sc_n + kt, :],
                    rhs=ats[kt][:, o:NTG],
                    start=(kt == 0),
                    stop=(kt == nk - 1),
                )
            # the denominator row lives in a score-pool slot (row 0 only):
            # no dedicated psum bank, which buys the 4-deep score rotation
            spt = sc_ps.tile([PART, NTG], F32, name="spt", tag="scp")
            sp = spt[0:1, :]
            nc.tensor.matmul(sp, lhsT=ones_sb, rhs=at_sum, start=True, stop=True)
            r = sm_p.tile([1, NTG], F32, name="r", tag="r", bufs=2)
            nc.vector.reciprocal_approx_fast(out=r, in_=sp)
            rb = sm_p.tile([PART, NTG], F32, name="rb", tag="rb", bufs=2)
            nc.gpsimd.partition_broadcast(rb, r)
            return (op, rb, h, q0, bb)

        def emit_D(op, rb, h, q0, bb):
            nc.vector.tensor_tensor(
                out=aoTs[bb][h][:, q0 - bb * s : q0 - bb * s + NTG],
                in0=op,
                in1=rb,
                op=A.mult,
            )

        pend = {"B": None, "D": None}
        d_count = [0, 0]  # normalizes emitted per batch

        def emit_D_pend():
            if pend["D"] is not None:
                d_count[pend["D"][4]] += 1
                emit_D(*pend["D"])
                pend["D"] = None

        def attn_iteration(bb, g, h):
            emit_D_pend()
            if pend["B"] is not None:
                pend["D"] = emit_B(*pend["B"])
                pend["B"] = None
            pend["B"] = emit_A(bb, g=g, h=h)

        def attn_flush():
            emit_D_pend()
            pend["D"] = emit_B(*pend["B"])
            pend["B"] = None
            emit_D_pend()

        # -------- R0 + R1: QKV over 2-bank psum pairs (kc-inner alternation
        # keeps consecutive matmuls on different banks).  R1 (batch-1 groups)
        # interleaves batch-0 attention between pair passes; the serializing
        # scalar-engine exp chain hides under the projection matmuls.
        b0_iters = [(g, h) for g in range(gq_n) for h in range(hpc)]
        b0_pos = 0
        qkv2_ctx = ExitStack()
        qkv2 = qkv2_ctx.enter_context(tc.tile_pool(name="qkv2", bufs=2, space="PSUM"))
        pairs = [(m_n - 1, m_n - 2), (0, 1), (2, 3)]  # (v,k), (q0,q1), (q2,q3)
        for ng in range(ng_n):
            t0 = ng * NTG
            hts = hts0 if ng == 0 else [ht_quarter(t0, kb) for kb in range(KCB)]
            attn_here = ng >= ng_n // 2
            for pi, pair in enumerate(pairs):
                ps_pair = [
                    qkv2.tile([PART, NTG], F32, name=f"qkvp{m}", tag="qp") for m in pair
                ]
                for kc in range(kc_n):
                    for i, m in enumerate(pair):
                        nc.tensor.matmul(
                            ps_pair[i],
                            lhsT=wq_sb[:, kc, m, :],
                            rhs=hts[kc // kcw][:, kc % kcw, :],
                            start=(kc == 0),
                            stop=(kc == kc_n - 1),
                        )
                xcs = [clip_ps(p) for p in ps_pair]
                for i, m in enumerate(pair):
                    if m == m_n - 1:
                        do_v(xcs[i], ng, qkv2, "qp")
                    else:
                        do_rope(xcs[i], m, t0)
                if attn_here:
                    for _ in range(1 if pi < 2 else 2):
                        if b0_pos < len(b0_iters):
                            g, h = b0_iters[b0_pos]
                            attn_iteration(0, g, h)
                            b0_pos += 1
        while b0_pos < len(b0_iters):
            g, h = b0_iters[b0_pos]
            attn_iteration(0, g, h)
            b0_pos += 1
        qkv2_ctx.close()
        p1.close()  # release weights / tables / activation pools

        # -------- R2: batch-1 attention (g-outer) with the output projection
        # interleaved; out-proj token chunks unlock as groups normalize
        o3_ps = ctx.enter_context(tc.tile_pool(name="o3ps", bufs=2, space="PSUM"))
        lp = ctx.enter_context(tc.tile_pool(name="late", bufs=1))
        aoTs[1] = [
            lp.tile([PART, s], BF16, name=f"aoT1{h}", tag=f"aoT1{h}") for h in range(hpc)
        ]
        wout_sb = lp.tile([PART, hpc, d], BF16, name="wout_sb", tag="wout")
        o3_sb = ctx.enter_context(tc.tile_pool(name="o3sb", bufs=3))
        for hc in range(hpc):
            nc.sync.dma_start(out=wout_sb[:, hc, :], in_=woutT[:, hc, :])

        op_state = {"uidx": 0, "obt": None}
        htch = tch_n // 2

        def unlocked_tch():
            if d_count[0] < gq_n * hpc:
                return (d_count[0] // hpc) * upg
            return htch + (d_count[1] // hpc) * upg

        def emit_op_units(nmax, tch_limit, drain=False):
            """out-proj units (tch, dgi): 4 accumulating matmuls + copy;
            one DMA out per completed tch row.  In the final drain the
            attention psum banks are idle, so rotate them in for slack."""
            n = 0
            while n < nmax and op_state["uidx"] < tch_n * dg_n:
                tch, dgi = divmod(op_state["uidx"], dg_n)
                if tch >= tch_limit:
                    return
                ao = aoTs[0] if tch < htch else aoTs[1]
                t0l = (tch % htch) * PART
                if dgi == 0:
                    op_state["obt"] = o3_sb.tile(
                        [PART, dg_n, NTG], BF16, name="obt", tag="obt"
                    )
                if drain and n % 5 >= 2:
                    pool, ptag = (sc_ps, "scp") if n % 5 < 4 else (o_ps, "op")
                else:
                    pool, ptag = o3_ps, "o3p"
                ps3 = pool.tile([PART, NTG], F32, name="o3p", tag=ptag)
                for hc in range(hpc):
                    nc.tensor.matmul(
                        ps3,
                        lhsT=ao[hc][:, t0l : t0l + PART],
                        rhs=wout_sb[:, hc, dgi * NTG : (dgi + 1) * NTG],
                        start=(hc == 0),
                        stop=(hc == hpc - 1),
                    )
                obt = op_state["obt"]
                if dgi % 2 == 0:
                    nc.scalar.activation(out=obt[:, dgi, :], in_=ps3, func=ACT.Copy)
                else:
                    nc.vector.tensor_copy(out=obt[:, dgi, :], in_=ps3)
                if dgi == dg_n - 1:
                    nc.gpsimd.dma_start(
                        out=outp[tch * PART : tch * PART + PART, :, :], in_=obt[:, :, :]
                    )
                op_state["uidx"] += 1
                n += 1

        for g in range(gq_n):
            for h in range(hpc):
                emit_op_units(16 if g == gq_n - 1 else 12, unlocked_tch())
                attn_iteration(1, g, h)
        attn_flush()
        emit_op_units(tch_n * dg_n, tch_n, drain=True)

    nc.finalize()
    return nc


def _host_prep(hidden_states, Wqkv, Wout, cos, sin, b=B, s=S, d=D, hpc=HPC, ncores=NCORES):
    """Build the per-core input maps (all bf16, pre-tiled layouts)."""
    t = b * s
    kc_n = d // PART
    m_n = hpc + 2
    hid = np.ascontiguousarray(hidden_states.reshape(t, d).T).astype(NPBF16)
    hid = hid.reshape(kc_n, PART, t)

    cosT = np.ascontiguousarray(cos.T).astype(NPBF16)
    st = sin.T.copy()
    st[: PART // 2] = -st[: PART // 2]
    sinTs = np.ascontiguousarray(st).astype(NPBF16)

    p = np.arange(PART)[:, None]
    j = np.arange(PART)[None, :]
    tri = (p <= j).astype(NPBF16)
    ident = np.eye(PART, dtype=NPBF16)

    in_maps = []
    for c in range(ncores):
        qrows = Wqkv[c * hpc * PART : (c + 1) * hpc * PART]
        krow = Wqkv[d + c * PART : d + (c + 1) * PART]
        vrow = Wqkv[d + (Wqkv.shape[0] - d) // 2 + c * PART :
                    d + (Wqkv.shape[0] - d) // 2 + (c + 1) * PART]
        Wc = np.concatenate([qrows, krow, vrow], axis=0)  # [m_n*128, d]
        wqkvT = np.ascontiguousarray(
            Wc.reshape(m_n, PART, kc_n, PART).transpose(3, 2, 0, 1)
        ).astype(NPBF16)
        woutT = np.ascontiguousarray(
            Wout[:, c * hpc * PART : (c + 1) * hpc * PART].T.reshape(hpc, PART, d).transpose(1, 0, 2)
        ).astype(NPBF16)
        in_maps.append(
            {
                "hidT": hid,
                "wqkvT": wqkvT,
                "cosT": cosT,
                "sinTs": sinTs,
                "tri": tri,
                "ident": ident,
                "woutT": woutT,
            }
        )
    return in_maps


_PROGRAM_CACHE = {}


def _get_program():
    key = (B, S, D, HPC)
    if key not in _PROGRAM_CACHE:
        _PROGRAM_CACHE[key] = _build_core_program()
    return _PROGRAM_CACHE[key]


def kernel(**inputs):
    import os

    from concourse.bass_utils import run_bass_kernel_spmd

    if os.environ.get("BASS_TRACE"):
        # tracing needs antenv.axon_hooks (absent in some images); if it's
        # missing and no shim was installed, force the untraced path rather
        # than crashing inside run_bass_kernel_spmd.
        try:
            import antenv.axon_hooks  # noqa: F401
        except ImportError:
            os.environ["BASS_NEVER_TRACE"] = "1"

    hs = np.asarray(inputs["hidden_states"], dtype=np.float32)
    Wqkv = np.asarray(inputs["Wqkv"], dtype=np.float32)
    Wout = np.asarray(inputs["Wout"], dtype=np.float32)
    cos = np.asarray(inputs["cos"], dtype=np.float32)
    sin = np.asarray(inputs["sin"], dtype=np.float32)

    in_maps = _host_prep(hs, Wqkv, Wout, cos, sin)
    nc = _get_program()
    res = run_bass_kernel_spmd(nc, in_maps, core_ids=list(range(NCORES)))
    STATS["exec_time_ns"] = res.exec_time_ns
    STATS["mean_exec_time_ns"] = res.mean_exec_time_ns
    STATS["trace"] = res.instructions_and_trace[1] if res.instructions_and_trace else None

    out = np.zeros((B * S, D), dtype=np.float32)
    for r in res.results:
        out += r["out"].astype(np.float32).reshape(B * S, D)
    return out.reshape(B, S, D)


# revision 48
# speedup vs baseline: 1.0040x; 1.0040x over previous
"""DBRX attention block on 8 Trainium2 NeuronCores.

Sharding: tensor-parallel over heads. Each core owns 4 query heads and the
single KV head that serves them (GQA group), computes the fused QKV
projection for its rows, clip, RoPE, causal flash-style attention, and a
full-width partial of the output projection (its 512 columns of the out-proj
contraction). The 8 partial outputs are summed on the host.

All matmuls run in bf16 (fp32 matmul is 4 cycles/row on TRN2 PE; bf16 is 1).
Softmax runs without max-subtraction (scores are O(1) for this input
distribution; exp cannot overflow), which matches the reference softmax
mathematically.

v5: the kernel is one software-pipelined pass, engine-balanced:
  R0: QKV for batch 0 (PE-bound; DMA preamble interleaved so the first
      matmul issues ~3us in).
  R1: QKV for batch 1 (2-bank psum pairs) interleaved with batch-0
      attention, whose serializing scalar-engine exp hides under the
      projection matmuls.
  R2: batch-1 attention interleaved with the output projection; out-proj
      tiles unlock progressively as query groups are normalized (g-outer
      loop), draining on whatever PE slack the exp pipeline leaves.
Other key structure:
  - causal trimming: the kt chunk at diagonal index u only computes
    q columns [128u, 512) in scores / exp / AV; the triangle mask multiply
    only touches the 128-wide partial block.
  - per-chunk softmax row-sums accumulate on the DVE (at_sum += at), one
    ones-matmul per (b, h, qgroup) turns them into denominators.
  - psum->sbuf drains alternate scalar/vector; out rows DMA once per
    128-token chunk (gpsimd has no PSUM port, so drains stay off it).

Layouts (per core):
  hidT    [KC, 128, T]        hidden states transposed, bf16
  wqkvT   [128, KC, 6, 128]   [d%128, d//128, row-block, row%128]; row blocks
                              0-3 = q heads, 4 = k head, 5 = v head
  cosT    [128, S]            rope cos, transposed (same for both batches)
  sinTs   [128, S]            rope sin, transposed, first 64 rows negated
  tri     [128, 128]          causal triangle mask, tri[p, j] = (p <= j)
  ident   [128, 128]          identity for PE transpose
  woutT   [128, 4, D]         Wout[:, core cols].T tiled by head chunk
  out     [T, 8, 512]         partial output (bf16), summed on host
"""

import sys

sys.path.insert(0, "/opt/trn_rl_repo")

import numpy as np
import ml_dtypes

import concourse.bass as bass
import concourse.tile as tile
from concourse import bacc, mybir
from contextlib import ExitStack

BF16 = mybir.dt.bfloat16
F32 = mybir.dt.float32
NPBF16 = ml_dtypes.bfloat16

# problem dims (must match reference.py / spec.json)
B, S, D = 2, 2048, 4096
NH, NKV, HD = 32, 8, 128
CLIP = 8.0
SCALE = HD**-0.5
NCORES = 8
HPC = NH // NCORES  # q heads per core

PART = 128
NTG = 512  # token-group width (phase-1 N, phase-2 qt group, phase-3 dout group)

STATS = {}


def _build_core_program(b=B, s=S, d=D, hpc=HPC, debug=False):
    """Bass program for ONE core (SPMD: same program, per-core data)."""
    t = b * s
    kc_n = d // PART  # contraction chunks
    m_n = hpc + 2  # qkv row blocks per core
    ng_n = t // NTG  # token groups
    sc_n = s // PART  # kt chunks per batch
    gq_n = s // NTG  # qt groups per batch
    dg_n = d // NTG  # out-proj dout groups
    tch_n = t // PART  # token chunks
    upg = NTG // PART  # diagonal sub-chunks per q group

    nc = bacc.Bacc()
    hidT = nc.declare_dram_parameter("hidT", [kc_n, PART, t], BF16, False)
    wqkvT = nc.declare_dram_parameter("wqkvT", [PART, kc_n, m_n, PART], BF16, False)
    cosT = nc.declare_dram_parameter("cosT", [PART, s], BF16, False)
    sinTs = nc.declare_dram_parameter("sinTs", [PART, s], BF16, False)
    tri = nc.declare_dram_parameter("tri", [PART, PART], BF16, False)
    ident = nc.declare_dram_parameter("ident", [PART, PART], BF16, False)
    woutT = nc.declare_dram_parameter("woutT", [PART, hpc, d], BF16, False)
    outp = nc.declare_dram_parameter("out", [t, dg_n, NTG], BF16, True)

    A = mybir.AluOpType
    ACT = mybir.ActivationFunctionType

    with tile.TileContext(nc) as tc, ExitStack() as ctx:
        persist = ctx.enter_context(tc.tile_pool(name="persist", bufs=1))
        qT = [persist.tile([PART, t], BF16, name=f"qT{h}", tag=f"qT{h}") for h in range(hpc)]
        kT = persist.tile([PART, t], BF16, name="kT", tag="kT")
        vsb = persist.tile([PART, tch_n, PART], BF16, name="vsb", tag="vsb")
        tri_sb = persist.tile([PART, PART], BF16, name="tri_sb", tag="tri")
        id_sb = persist.tile([PART, PART], BF16, name="id_sb", tag="ident")
        ones_sb = persist.tile([PART, 1], BF16, name="ones_sb", tag="ones")

        nc.vector.memset(ones_sb, 1.0)

        # attention pools open below the QKV pools on the pool stack so the
        # QKV pools can release mid-program (pools are strictly LIFO)
        sc_ps = ctx.enter_context(tc.tile_pool(name="scps", bufs=4, space="PSUM"))
        o_ps = ctx.enter_context(tc.tile_pool(name="ops", bufs=2, space="PSUM"))
        at_p = ctx.enter_context(tc.tile_pool(name="atp", bufs=22))
        sm_p = ctx.enter_context(tc.tile_pool(name="smp", bufs=3))
        ao_p = ctx.enter_context(tc.tile_pool(name="aop", bufs=1))
        aoTs = [
            [ao_p.tile([PART, s], BF16, name=f"aoT0{h}", tag=f"aoT0{h}") for h in range(hpc)],
            None,  # batch 1 tiles allocated in R2, after QKV pools release
        ]

        # ---------------- QKV machinery
        p1 = ExitStack()  # closed after R1 (frees weights/tables/activations)
        wp = p1.enter_context(tc.tile_pool(name="wp", bufs=1))
        wq_sb = wp.tile([PART, kc_n, m_n, PART], BF16, name="wq_sb", tag="wq")
        cs = p1.enter_context(tc.tile_pool(name="cs", bufs=1))
        cos_sb = cs.tile([PART, s], BF16, name="cos_sb", tag="cos")
        sin_sb = cs.tile([PART, s], BF16, name="sin_sb", tag="sin")
        hid_pool = p1.enter_context(tc.tile_pool(name="hidp", bufs=5))
        ev = p1.enter_context(tc.tile_pool(name="ev", bufs=3))

        KCB = 4  # hid quarter-tiles per token group
        kcw = kc_n // KCB

        def ht_quarter(t0, kb):
            q = hid_pool.tile([PART, kcw, NTG], BF16, name="ht", tag="ht")
            nc.sync.dma_start(
                out=q,
                in_=hidT[kb * kcw : (kb + 1) * kcw, :, t0 : t0 + NTG].transpose(
                    [1, 0, 2]
                ),
            )
            return q

        def clip_ps(ps):
            xc = ev.tile([PART, NTG], BF16, name="xc", tag="xc", bufs=5)
            nc.vector.tensor_scalar(
                out=xc, in0=ps, scalar1=CLIP, scalar2=-CLIP, op0=A.min, op1=A.max
            )
            return xc

        def do_v(xc, ng, pool, tag):
            for u in range(upg):
                tp = pool.tile([PART, PART], BF16, name="tp", tag=tag)
                nc.tensor.transpose(tp, xc[:, u * PART : (u + 1) * PART], id_sb)
                nc.scalar.copy(out=vsb[:, ng * upg + u, :], in_=tp)

        def do_rope(xc, m, t0):
            tl = t0 % s  # cos/sin tables are per-batch
            rot = ev.tile([PART, NTG], BF16, name="rot", tag="rot")
            hh = PART // 2
            nc.gpsimd.dma_start(out=rot[0:hh, :], in_=xc[hh:PART, :])
            nc.gpsimd.dma_start(out=rot[hh:PART, :], in_=xc[0:hh, :])
            t1 = ev.tile([PART, NTG], BF16, name="t1", tag="t1")
            nc.vector.tensor_tensor(
                out=t1, in0=xc, in1=cos_sb[:, tl : tl + NTG], op=A.mult
            )
            t2 = ev.tile([PART, NTG], BF16, name="t2", tag="t2")
            nc.vector.tensor_tensor(
                out=t2, in0=rot, in1=sin_sb[:, tl : tl + NTG], op=A.mult
            )
            dest = kT if m == m_n - 2 else qT[m]
            nc.vector.tensor_tensor(
                out=dest[:, t0 : t0 + NTG], in0=t1, in1=t2, op=A.add
            )

        # -------- preamble: first token group's activations interleaved with
        # weights so the first matmul issues ~3us in; tables land last
        # ht quarters and weight blocks ordered so ng0's first pair pass
        # (v,k) can begin early and never starves: v/k weight rows land
        # first, q rows while pass 1 streams
        hts0 = [ht_quarter(0, 0)]
        nc.sync.dma_start(out=wq_sb[:, :, hpc:, :], in_=wqkvT[:, :, hpc:, :])
        for kb in range(1, KCB):
            hts0.append(ht_quarter(0, kb))
        nc.sync.dma_start(out=wq_sb[:, :, 0:2, :], in_=wqkvT[:, :, 0:2, :])
        nc.sync.dma_start(out=wq_sb[:, :, 2:hpc, :], in_=wqkvT[:, :, 2:hpc, :])
        nc.sync.dma_start(out=tri_sb, in_=tri[:, :])
        nc.sync.dma_start(out=id_sb, in_=ident[:, :])
        nc.sync.dma_start(out=cos_sb, in_=cosT[:, :])
        nc.sync.dma_start(out=sin_sb, in_=sinTs[:, :])

        def col_off(kt, g):
            u = kt - g * upg
            return u * PART if u > 0 else 0

        def emit_A(bb, h, g):
            """scores + exp + mask + running row-total for one q group"""
            q0 = bb * s + g * NTG
            nk = (g + 1) * upg
            ats = []
            at_sum = sm_p.tile([PART, NTG], BF16, name="at_sum", tag="at_sum", bufs=3)
            for kt in range(nk):
                o = col_off(kt, g)
                scp = sc_ps.tile([PART, NTG], F32, name="scp", tag="scp")
                nc.tensor.matmul(
                    scp[:, o:NTG],
                    lhsT=kT[:, bb * s + kt * PART : bb * s + (kt + 1) * PART],
                    rhs=qT[h][:, q0 + o : q0 + NTG],
                    start=True,
                    stop=True,
                )
                at = at_p.tile([PART, NTG], BF16, name="at", tag="at")
                nc.scalar.activation(
                    out=at[:, o:NTG], in_=scp[:, o:NTG], func=ACT.Exp, scale=SCALE
                )
                u = kt - g * upg
                if u >= 0:  # diagonal chunk: triangle on its partial block
                    nc.vector.tensor_tensor(
                        out=at[:, o : o + PART],
                        in0=at[:, o : o + PART],
                        in1=tri_sb,
                        op=A.mult,
                    )
                if kt == 0:
                    nc.vector.tensor_copy(out=at_sum, in_=at)
                else:
                    nc.vector.tensor_tensor(
                        out=at_sum[:, o:NTG],
                        in0=at_sum[:, o:NTG],
                        in1=at[:, o:NTG],
                        op=A.add,
                    )
                ats.append(at)
            return (bb, h, g, q0, nk, ats, at_sum)

        def emit_B(bb, h, g, q0, nk, ats, at_sum):
            """AV accumulation + single row-sum matmul + reciprocal"""
            op = o_ps.tile([PART, NTG], F32, name="op", tag="op")
            for kt in range(nk):
                o = col_off(kt, g)
                nc.tensor.matmul(
                    op[:, o:NTG],
                    lhsT=vsb[:, bb * sc_n + kt, :],
                    rhs=ats[kt][:, o:NTG],
                    start=(kt == 0),
                    stop=(kt == nk - 1),
                )
            # the denominator row lives in a score-pool slot (row 0 only):
            # no dedicated psum bank, which buys the 4-deep score rotation
            spt = sc_ps.tile([PART, NTG], F32, name="spt", tag="scp")
            sp = spt[0:1, :]
            nc.tensor.matmul(sp, lhsT=ones_sb, rhs=at_sum, start=True, stop=True)
            r = sm_p.tile([1, NTG], F32, name="r", tag="r", bufs=2)
            nc.vector.reciprocal_approx_fast(out=r, in_=sp)
            rb = sm_p.tile([PART, NTG], F32, name="rb", tag="rb", bufs=2)
            nc.gpsimd.partition_broadcast(rb, r)
            return (op, rb, h, q0, bb)

        def emit_D(op, rb, h, q0, bb):
            nc.vector.tensor_tensor(
                out=aoTs[bb][h][:, q0 - bb * s : q0 - bb * s + NTG],
                in0=op,
                in1=rb,
                op=A.mult,
            )

        pend = {"B": None, "D": None}
        d_count = [0, 0]  # normalizes emitted per batch

        def emit_D_pend():
            if pend["D"] is not None:
                d_count[pend["D"][4]] += 1
                emit_D(*pend["D"])
                pend["D"] = None

        def attn_iteration(bb, g, h):
            emit_D_pend()
            if pend["B"] is not None:
                pend["D"] = emit_B(*pend["B"])
                pend["B"] = None
            pend["B"] = emit_A(bb, g=g, h=h)

        def attn_flush():
            emit_D_pend()
            pend["D"] = emit_B(*pend["B"])
            pend["B"] = None
            emit_D_pend()

        # -------- R0 + R1: QKV over 2-bank psum pairs (kc-inner alternation
        # keeps consecutive matmuls on different banks).  R1 (batch-1 groups)
        # interleaves batch-0 attention between pair passes; the serializing
        # scalar-engine exp chain hides under the projection matmuls.
        b0_iters = [(g, h) for g in range(gq_n) for h in range(hpc)]
        b0_pos = 0
        qkv2_ctx = ExitStack()
        qkv2 = qkv2_ctx.enter_context(tc.tile_pool(name="qkv2", bufs=2, space="PSUM"))
        pairs = [(m_n - 1, m_n - 2), (0, 1), (2, 3)]  # (v,k), (q0,q1), (q2,q3)
        for ng in range(ng_n):
            t0 = ng * NTG
            hts = hts0 if ng == 0 else [ht_quarter(t0, kb) for kb in range(KCB)]
            attn_here = ng >= ng_n // 2
            for pi, pair in enumerate(pairs):
                ps_pair = [
                    qkv2.tile([PART, NTG], F32, name=f"qkvp{m}", tag="qp") for m in pair
                ]
                for kc in range(kc_n):
                    for i, m in enumerate(pair):
                        nc.tensor.matmul(
                            ps_pair[i],
                            lhsT=wq_sb[:, kc, m, :],
                            rhs=hts[kc // kcw][:, kc % kcw, :],
                            start=(kc == 0),
                            stop=(kc == kc_n - 1),
                        )
                xcs = [clip_ps(p) for p in ps_pair]
                for i, m in enumerate(pair):
                    if m == m_n - 1:
                        do_v(xcs[i], ng, qkv2, "qp")
                    else:
                        do_rope(xcs[i], m, t0)
                if attn_here:
                    for _ in range(1 if pi < 2 else 2):
                        if b0_pos < len(b0_iters):
                            g, h = b0_iters[b0_pos]
                            attn_iteration(0, g, h)
                            b0_pos += 1
        while b0_pos < len(b0_iters):
            g, h = b0_iters[b0_pos]
            attn_iteration(0, g, h)
            b0_pos += 1
        qkv2_ctx.close()
        p1.close()  # release weights / tables / activation pools

        # -------- R2: batch-1 attention (g-outer) with the output projection
        # interleaved; out-proj token chunks unlock as groups normalize
        o3_ps = ctx.enter_context(tc.tile_pool(name="o3ps", bufs=2, space="PSUM"))
        lp = ctx.enter_context(tc.tile_pool(name="late", bufs=1))
        aoTs[1] = [
            lp.tile([PART, s], BF16, name=f"aoT1{h}", tag=f"aoT1{h}") for h in range(hpc)
        ]
        wout_sb = lp.tile([PART, hpc, d], BF16, name="wout_sb", tag="wout")
        o3_sb = ctx.enter_context(tc.tile_pool(name="o3sb", bufs=3))
        for hc in range(hpc):
            nc.sync.dma_start(out=wout_sb[:, hc, :], in_=woutT[:, hc, :])

        op_state = {"uidx": 0, "obt": None}
        htch = tch_n // 2

        def unlocked_tch():
            if d_count[0] < gq_n * hpc:
                return (d_count[0] // hpc) * upg
            return htch + (d_count[1] // hpc) * upg

        def emit_op_units(nmax, tch_limit, drain=False):
            """out-proj units (tch, dgi): 4 accumulating matmuls + copy;
            one DMA out per completed tch row.  In the final drain the
            attention psum banks are idle, so rotate them in for slack."""
            n = 0
            while n < nmax and op_state["uidx"] < tch_n * dg_n:
                tch, dgi = divmod(op_state["uidx"], dg_n)
                if tch >= tch_limit:
                    return
                ao = aoTs[0] if tch < htch else aoTs[1]
                t0l = (tch % htch) * PART
                if dgi == 0:
                    op_state["obt"] = o3_sb.tile(
                        [PART, dg_n, NTG], BF16, name="obt", tag="obt"
                    )
                if drain and n % 5 >= 2:
                    pool, ptag = (sc_ps, "scp") if n % 5 < 4 else (o_ps, "op")
                else:
                    pool, ptag = o3_ps, "o3p"
                ps3 = pool.tile([PART, NTG], F32, name="o3p", tag=ptag)
                for hc in range(hpc):
                    nc.tensor.matmul(
                        ps3,
                        lhsT=ao[hc][:, t0l : t0l + PART],
                        rhs=wout_sb[:, hc, dgi * NTG : (dgi + 1) * NTG],
                        start=(hc == 0),
                        stop=(hc == hpc - 1),
                    )
                obt = op_state["obt"]
                if dgi % 2 == 0:
                    nc.scalar.activation(out=obt[:, dgi, :], in_=ps3, func=ACT.Copy)
                else:
                    nc.vector.tensor_copy(out=obt[:, dgi, :], in_=ps3)
                if dgi == dg_n - 1:
                    nc.gpsimd.dma_start(
                        out=outp[tch * PART : tch * PART + PART, :, :], in_=obt[:, :, :]
                    )
                op_state["uidx"] += 1
                n += 1

        for g in range(gq_n):
            for h in range(hpc):
                emit_op_units(16 if g == gq_n - 1 else 12, unlocked_tch())
                attn_iteration(1, g, h)
        attn_flush()
        emit_op_units(tch_n * dg_n, tch_n, drain=True)

    nc.finalize()
    return nc


def _host_prep(hidden_states, Wqkv, Wout, cos, sin, b=B, s=S, d=D, hpc=HPC, ncores=NCORES):
    """Build the per-core input maps (all bf16, pre-tiled layouts)."""
    t = b * s
    kc_n = d // PART
    m_n = hpc + 2
    hid = np.ascontiguousarray(hidden_states.reshape(t, d).T).astype(NPBF16)
    hid = hid.reshape(kc_n, PART, t)

    cosT = np.ascontiguousarray(cos.T).astype(NPBF16)
    st = sin.T.copy()
    st[: PART // 2] = -st[: PART // 2]
    sinTs = np.ascontiguousarray(st).astype(NPBF16)

    p = np.arange(PART)[:, None]
    j = np.arange(PART)[None, :]
    tri = (p <= j).astype(NPBF16)
    ident = np.eye(PART, dtype=NPBF16)

    in_maps = []
    for c in range(ncores):
        qrows = Wqkv[c * hpc * PART : (c + 1) * hpc * PART]
        krow = Wqkv[d + c * PART : d + (c + 1) * PART]
        vrow = Wqkv[d + (Wqkv.shape[0] - d) // 2 + c * PART :
                    d + (Wqkv.shape[0] - d) // 2 + (c + 1) * PART]
        Wc = np.concatenate([qrows, krow, vrow], axis=0)  # [m_n*128, d]
        wqkvT = np.ascontiguousarray(
            Wc.reshape(m_n, PART, kc_n, PART).transpose(3, 2, 0, 1)
        ).astype(NPBF16)
        woutT = np.ascontiguousarray(
            Wout[:, c * hpc * PART : (c + 1) * hpc * PART].T.reshape(hpc, PART, d).transpose(1, 0, 2)
        ).astype(NPBF16)
        in_maps.append(
            {
                "hidT": hid,
                "wqkvT": wqkvT,
                "cosT": cosT,
                "sinTs": sinTs,
                "tri": tri,
                "ident": ident,
                "woutT": woutT,
            }
        )
    return in_maps


_PROGRAM_CACHE = {}


def _get_program():
    key = (B, S, D, HPC)
    if key not in _PROGRAM_CACHE:
        _PROGRAM_CACHE[key] = _build_core_program()
    return _PROGRAM_CACHE[key]


def kernel(**inputs):
    import os

    from concourse.bass_utils import run_bass_kernel_spmd

    if os.environ.get("BASS_TRACE"):
        # tracing needs antenv.axon_hooks (absent in some images); if it's
        # missing and no shim was installed, force the untraced path rather
        # than crashing inside run_bass_kernel_spmd.
        try:
            import antenv.axon_hooks  # noqa: F401
        except ImportError:
            os.environ["BASS_NEVER_TRACE"] = "1"

    hs = np.asarray(inputs["hidden_states"], dtype=np.float32)
    Wqkv = np.asarray(inputs["Wqkv"], dtype=np.float32)
    Wout = np.asarray(inputs["Wout"], dtype=np.float32)
    cos = np.asarray(inputs["cos"], dtype=np.float32)
    sin = np.asarray(inputs["sin"], dtype=np.float32)

    in_maps = _host_prep(hs, Wqkv, Wout, cos, sin)
    nc = _get_program()
    res = run_bass_kernel_spmd(nc, in_maps, core_ids=list(range(NCORES)))
    STATS["exec_time_ns"] = res.exec_time_ns
    STATS["mean_exec_time_ns"] = res.mean_exec_time_ns
    STATS["trace"] = res.instructions_and_trace[1] if res.instructions_and_trace else None

    out = np.zeros((B * S, D), dtype=np.float32)
    for r in res.results:
        out += r["out"].astype(np.float32).reshape(B * S, D)
    return out.reshape(B, S, D)
